# revision 1
# baseline (speedup 1.0000x reference)
"""Trainium2 Bass kernel for nn_CosSimSpatTempConvNet.

Math (reference):
  merged[f,c,k] = conv_w[f,k] * spat_w[f,c]                  (rank-1 kernel)
  conved[b,f,t] = sum_{c,k} merged[f,c,k] * x[b,c,t+k]       (valid conv, Tout=T-K+1)
  norm_w[f]    = ||conv_w[f]|| * ||spat_w[f]||
  norm_in[b,t] = sqrt(sum_{c,k} x[b,c,t+k]^2)
  cos[b,f,t]   = conved * 64 / (norm_w[f] * norm_in[b,t])
  out[b]       = sum_f (mean_t |cos[b,f,t]| * weight[f] + bias[f])

Device strategy (8 cores, data-parallel over batch, 8 b per core):
  * Full conv as TensorE matmuls with contraction dim 128 = (c, k2):
    x2 tile holds x[b] on partitions 0-63 and x[b] shifted by one time-step
    on partitions 64-127, so 32 PSUM-accumulated matmuls (one per k-pair)
    with moving-operand slices x2[:, t0+2*kp : ...] produce conved[:, t0:t0+512].
    Stationaries W2[kp][(c,k2), f] = conv_w[f, 2kp+k2]*spat_w[f,c] are
    host-precomputed (tiny) and passed as an input.
  * fp32 data, matmuls issued as float32r (full-rate for N>=256).
  * norm_in: x^2 on ScalarE, sum over c via ones-stationary matmul,
    sliding-window-64 sum via 6 doubling shift-adds on VectorE,
    reciprocal (VectorE) + sqrt (ScalarE).
  * epilogue per (b, t-tile): broadcast nrecip row across 128 partitions with
    a K=1 matmul, then one fused VectorE scalar_tensor_tensor:
    (|conved| via abs_max with 0) * nrecip, accumulated over t (accum_out).
  * finish: S[f,b] sums -> one matmul with stationary g[f] = 64*weight/(4033*norm_w)
    contracting over f, plus host-computed sum(bias).
"""

import contextlib
import ctypes
import sys
import types

import numpy as np

import concourse.bass as bass
import concourse.mybir as mybir
import concourse.tile as tile
from concourse.bass_utils import run_bass_kernel_spmd
from concourse.vector_clock import ScopedClock

F32 = mybir.dt.float32
F32R = mybir.dt.float32r

B, CIN, T = 64, 64, 4096
F, K = 128, 64
TOUT = T - K + 1          # 4033
NCORES = 8
BLOC = B // NCORES        # 8 batches per core
KP = K // 2               # 32 packed k-pairs
TS = 512                  # moving-operand tile (one fp32 PSUM bank)
NTILES = (TOUT + TS - 1) // TS      # 8 (last tile 449)
SCALE = 64.0              # sqrt(CIN*K)

AF = mybir.ActivationFunctionType
ALU = mybir.AluOpType


# ---------------------------------------------------------------------------
# Container fixups: walrus here rejects >1 sem-wait on a Drain; TileContext's
# tail drain carries one wait per logical processor.  Chunk into single-wait
# drains.  Also recreate the (absent) antenv.axon_hooks NTFF profile hook so
# trace=True works when a test harness wants timings.
# ---------------------------------------------------------------------------

def _patched_drain_and_barrier(self, tick_clock, wait_clock):
    nc = self.nc
    drain_inst = nc.sync.drain()
    wait_clock.add_sem_waits(
        drain_inst.ins, ScopedClock({None: tick_clock.global_clock})
    )
    si = drain_inst.ins.sync_info
    waits = list(si.on_wait or []) if si else []
    if len(waits) > 1:
        si.on_wait = waits[:1]
        for w in waits[1:]:
            d2 = nc.sync.drain()
            si2 = d2.ins.sync_info
            if si2 is None:
                d2.ins.sync_info = mybir.SyncInfo(on_wait=[w], on_update=[])
            else:
                si2.on_wait = [w]
    nc.all_engine_barrier()
    assert self.sems is not None
    popped = nc._tile_sem_poison_stack.pop()
    assert popped is self._sem_poison
    nc.clear_and_free_semaphores(list(self.sems.allocated().values()))
    nc.all_engine_barrier()


def _install_ntff_hook():
    if "antenv.axon_hooks" in sys.modules:
        return
    try:
        lib = ctypes.CDLL("/opt/axon/libaxon_pjrt.so")
    except OSError:
        return
    if not hasattr(lib, "axon_start_nrt_profile"):
        return
    lib.axon_start_nrt_profile.argtypes = [
        ctypes.POINTER(ctypes.c_int64),
        ctypes.c_size_t,
    ]
    lib.axon_start_nrt_profile.restype = ctypes.c_int64
    lib.axon_stop_nrt_profile.argtypes = [ctypes.c_char_p]
    lib.axon_stop_nrt_profile.restype = ctypes.c_int64

    @contextlib.contextmanager
    def _hook(output_dir, device_ids):
        import jax

        jax.devices()
        if device_ids:
            ids = (ctypes.c_int64 * len(device_ids))(*device_ids)
            rc = lib.axon_start_nrt_profile(ids, len(device_ids))
        else:
            rc = lib.axon_start_nrt_profile(None, 0)
        if rc != 0:
            raise RuntimeError(f"axon_start_nrt_profile rc={rc}")
        try:
            yield
        finally:
            n = lib.axon_stop_nrt_profile(str(output_dir).encode())
            print(f"profile: {n} ntff file(s) in {output_dir}", file=sys.stderr)

    mod = types.ModuleType("antenv.axon_hooks")
    mod.get_axon_ntff_profile_hook = lambda: _hook
    mod.set_axon_ntff_profile_hook = lambda h: None
    import antenv

    antenv.axon_hooks = mod
    sys.modules["antenv.axon_hooks"] = mod


_ORIG_COMMIT = tile.TileContext._commit_instruction


def _commit_split_waits(self, inst, lazy_reg_writes=True):
    """walrus here allows only one sem-wait per instruction; move extras
    onto same-engine NOPs committed immediately before the instruction."""
    si = getattr(inst, "sync_info", None)
    if (
        si is not None
        and si.on_wait
        and len(si.on_wait) > 1
        and inst.engine != mybir.EngineType.Unassigned
    ):
        waits = list(si.on_wait)
        si.on_wait = waits[:1]
        for i, w in enumerate(waits[1:]):
            nop = mybir.InstNoOp(
                name=f"{inst.name}-wsplit{i}", ins=[], outs=[]
            )
            nop.engine = inst.engine
            nop.sync_info = mybir.SyncInfo(on_wait=[w], on_update=[])
            _ORIG_COMMIT(self, nop, lazy_reg_writes=False)
    return _ORIG_COMMIT(self, inst, lazy_reg_writes)


def install_fixups():
    tile.TileContext._drain_and_barrier = _patched_drain_and_barrier
    tile.TileContext._commit_instruction = _commit_split_waits
    _install_ntff_hook()


# ---------------------------------------------------------------------------
# Device program (identical on all 8 cores; inputs differ per core)
# ---------------------------------------------------------------------------

def build_program() -> bass.Bass:
    install_fixups()
    nc = bass.Bass()

    xs_in = nc.dram_tensor("xs", [BLOC, CIN, T], F32, kind="ExternalInput")
    w2_in = nc.dram_tensor("w2", [128, KP, F], F32, kind="ExternalInput")
    g_in = nc.dram_tensor("g", [F, 1], F32, kind="ExternalInput")
    bsum_in = nc.dram_tensor("bsum", [1, 1], F32, kind="ExternalInput")
    ones64_in = nc.dram_tensor("ones64", [64, 1], F32, kind="ExternalInput")
    ones1_in = nc.dram_tensor("ones1", [1, 128], F32, kind="ExternalInput")
    zcol_in = nc.dram_tensor("zcol", [64, 1], F32, kind="ExternalInput")
    out_d = nc.dram_tensor("out", [1, BLOC], F32, kind="ExternalOutput")

    PS = bass.MemorySpace.PSUM

    with tile.TileContext(nc) as tc:
        with (
            tc.tile_pool(name="const", bufs=1) as constp,
            tc.tile_pool(name="xp", bufs=2) as xp,
            tc.tile_pool(name="sqp", bufs=2) as sqp,
            tc.tile_pool(name="bigp", bufs=3) as bigp,
            tc.tile_pool(name="rowp", bufs=3) as rowp,
            tc.tile_pool(name="scrp", bufs=2) as scrp,
            tc.tile_pool(name="accp", bufs=2) as accp,
            tc.tile_pool(name="pconv", bufs=3, space=PS) as pconv,
            tc.tile_pool(name="pbc", bufs=2, space=PS) as pbc,
            tc.tile_pool(name="psq", bufs=2, space=PS) as psq,
            tc.tile_pool(name="pfin", bufs=1, space=PS) as pfin,
        ):
            # constants.  Tiles feeding float32r matmuls are declared f32r so
            # every producer instruction emits "rounded" f32r (BIR verifier
            # requirement); DRAM stays f32 and DMAs bitcast the source.
            w2sb = constp.tile([128, KP, F], F32R)
            nc.sync.dma_start(w2sb[:], w2_in[:].bitcast(F32R))
            gsb = constp.tile([F, 1], F32)
            nc.sync.dma_start(gsb[:], g_in[:])
            bsumsb = constp.tile([1, 1], F32)
            nc.sync.dma_start(bsumsb[:], bsum_in[:])
            ones64 = constp.tile([64, 1], F32R)
            nc.sync.dma_start(ones64[:], ones64_in[:].bitcast(F32R))
            ones1 = constp.tile([1, 128], F32R)
            nc.sync.dma_start(ones1[:], ones1_in[:].bitcast(F32R))
            zcol = constp.tile([64, 1], F32R)
            nc.sync.dma_start(zcol[:], zcol_in[:].bitcast(F32R))

            sq_all = constp.tile([BLOC, T], F32)   # sum_c x^2, one row per b
            S = constp.tile([F, BLOC], F32)        # per-(f,b) |cos| sums

            # ---- phase 1: input norms ------------------------------------
            for b in range(BLOC):
                xt = xp.tile([128, T], F32R, tag="xt")
                nc.sync.dma_start(xt[0:64, :], xs_in[b].bitcast(F32R))
                xsq = sqp.tile([64, T], F32R, tag="xsq")
                nc.scalar.activation(xsq[:], xt[0:64, :], AF.Square)
                for ts in range(T // TS):
                    pq = psq.tile([1, TS], F32)
                    nc.tensor.matmul(
                        pq[:],
                        ones64[:],
                        xsq[:, ts * TS:(ts + 1) * TS],
                    )
                    sqrow = rowp.tile([1, TS], F32, tag="sqrow")
                    nc.scalar.copy(sqrow[:], pq[:])
                    # cross-partition row placement -> DMA, not DVE
                    nc.sync.dma_start(
                        sq_all[b:b + 1, ts * TS:(ts + 1) * TS], sqrow[:]
                    )

            # sliding-window-64 sum via doubling shifts, then 1/sqrt
            cur = sq_all
            width = T
            for sh in (1, 2, 4, 8, 16, 32):
                width -= sh
                nxt = bigp.tile([BLOC, T], F32, tag="slide")
                nc.vector.tensor_tensor(
                    nxt[:, 0:width], cur[:, 0:width], cur[:, sh:sh + width],
                    op=ALU.add,
                )
                cur = nxt
            assert width == TOUT
            recip_t = bigp.tile([BLOC, T], F32, tag="slide")
            nc.vector.reciprocal(recip_t[:, 0:TOUT], cur[:, 0:TOUT])
            nrec8 = constp.tile([BLOC, T], F32R)
            nc.scalar.activation(nrec8[:, 0:TOUT], recip_t[:, 0:TOUT], AF.Sqrt)

            # ---- phase 2: conv + cosine epilogue -------------------------
            for b in range(BLOC):
                x2 = xp.tile([128, T], F32R, tag="xt")
                nc.sync.dma_start(x2[0:64, :], xs_in[b].bitcast(F32R))
                nc.sync.dma_start(x2[64:128, 0:T - 1], x2[0:64, 1:T])
                nc.sync.dma_start(x2[64:128, T - 1:T], zcol[:])
                acc = accp.tile([F, NTILES], F32)
                for ts in range(NTILES):
                    t0 = ts * TS
                    nt = min(TS, TOUT - t0)
                    nt_mm = nt + (nt & 1)   # f32r moving free size must be even
                    pc = pconv.tile([F, TS], F32)
                    for kp in range(KP):
                        nc.tensor.matmul(
                            pc[:, 0:nt_mm],
                            w2sb[:, kp, :],
                            x2[:, t0 + 2 * kp: t0 + 2 * kp + nt_mm],
                            start=(kp == 0),
                            stop=(kp == KP - 1),
                        )
                    nrow = rowp.tile([1, TS], F32R)
                    nc.sync.dma_start(nrow[0:1, 0:nt], nrec8[b:b + 1, t0:t0 + nt])
                    if nt_mm != nt:
                        nc.sync.dma_start(nrow[0:1, nt:nt_mm], zcol[0:1, :].bitcast(F32R))
                    pb = pbc.tile([128, TS], F32)
                    nc.tensor.matmul(
                        pb[:, 0:nt_mm],
                        ones1[:],
                        nrow[0:1, 0:nt_mm],
                    )
                    # DVE may read only one PSUM operand: take |conv| on
                    # ScalarE (PSUM->SBUF), then fuse multiply+row-sum on DVE.
                    scr = scrp.tile([F, TS], F32)
                    nc.scalar.activation(scr[:, 0:nt], pc[:, 0:nt], AF.Abs)
                    nc.vector.scalar_tensor_tensor(
                        scr[:, 0:nt],
                        scr[:, 0:nt],
                        1.0,
                        pb[:, 0:nt],
                        op0=ALU.mult,
                        op1=ALU.mult,
                        accum_out=acc[:, ts:ts + 1],
                    )
                nc.vector.reduce_sum(
                    S[:, b:b + 1], acc[:], axis=mybir.AxisListType.X
                )

            # ---- finish: out[b] = sum_f g[f]*S[f,b] + sum_f bias[f] ------
            pf = pfin.tile([1, BLOC], F32)
            nc.tensor.matmul(pf[:], gsb[:], S[:])
            out_sb = constp.tile([1, BLOC], F32)
            nc.scalar.add(out_sb[:], pf[:], bsumsb[0:1, 0:1])
            nc.sync.dma_start(out_d[:], out_sb[:])

    return nc


_PROGRAM: bass.Bass | None = None


def _get_program() -> bass.Bass:
    global _PROGRAM
    if _PROGRAM is None:
        _PROGRAM = build_program()
    return _PROGRAM


# ---------------------------------------------------------------------------
# Host entry point
# ---------------------------------------------------------------------------

def host_params(conv_weights, spat_weights, weight, bias):
    """Tiny host-side precomputation of stationaries and scalars."""
    conv = np.asarray(conv_weights, dtype=np.float64)
    spat = np.asarray(spat_weights, dtype=np.float64)
    w = np.asarray(weight, dtype=np.float64)
    bb = np.asarray(bias, dtype=np.float64)

    # W2[k, c, f] = conv[f, k] * spat[f, c]; pack partition index (k2, c)
    W = np.einsum("fk,fc->kcf", conv, spat)            # [K, C, F]
    V = W.reshape(KP, 2, CIN, F).reshape(KP, 128, F)   # [(kp), (k2,c), F]
    w2 = np.ascontiguousarray(V.transpose(1, 0, 2)).astype(np.float32)

    norm_w = np.sqrt((spat * spat).sum(1) * (conv * conv).sum(1))  # [F]
    g = (SCALE / (TOUT * norm_w) * w).astype(np.float32).reshape(F, 1)
    bsum = np.array([[bb.sum()]], dtype=np.float32)
    return w2, g, bsum


def kernel(x, conv_weights, spat_weights, weight, bias):
    x = np.ascontiguousarray(np.asarray(x, dtype=np.float32))
    w2, g, bsum = host_params(conv_weights, spat_weights, weight, bias)

    nc = _get_program()
    in_maps = []
    for c in range(NCORES):
        in_maps.append(
            {
                "xs": np.ascontiguousarray(x[c * BLOC:(c + 1) * BLOC]),
                "w2": w2,
                "g": g,
                "bsum": bsum,
                "ones64": np.ones((64, 1), np.float32),
                "ones1": np.ones((1, 128), np.float32),
                "zcol": np.zeros((64, 1), np.float32),
            }
        )
    res = run_bass_kernel_spmd(nc, in_maps, core_ids=list(range(NCORES)))
    out = np.concatenate(
        [res.results[c]["out"].reshape(BLOC) for c in range(NCORES)]
    )
    return out.astype(np.float32)



# revision 3
# speedup vs baseline: 1.0907x; 1.0907x over previous
"""Trainium2 Bass kernel for nn_CosSimSpatTempConvNet — rank-1 factorized.

Math (reference):
  merged[f,c,k] = conv_w[f,k] * spat_w[f,c]                  (rank-1 kernel)
  conved[b,f,t] = sum_{c,k} merged[f,c,k] * x[b,c,t+k]       (valid conv, Tout=4033)
  norm_w[f]    = ||conv_w[f]|| * ||spat_w[f]||
  norm_in[b,t] = sqrt(sum_{c,k} x[b,c,t+k]^2)
  cos[b,f,t]   = conved * 64 / (norm_w[f] * norm_in[b,t])
  out[b]       = sum_f (mean_t |cos[b,f,t]| * weight[f] + bias[f])

Key idea vs v1: exploit the rank-1 structure (32x fewer MACs).
  conved[b,f,t] = sum_k conv_w[f,k] * y[b,f,t+k],  y = spat_w @ x
Stage 1 (spatial matmul) puts TIME on PSUM partitions:
  per 128-wide chunk c: stationary = x[b][:, 128c:128c+128] (64x128 bf16),
  moving = spat_w^T (64x128) -> y_chunk[i, f] = y[b, f, 128c+i].
Stage 2 (temporal depthwise conv) is a Toeplitz-stationary matmul:
  out_f[64m+j] = sum_i A_f[i, j] * y-window,  A_f[i, j] = w_f[i-j].
  Even blocks (m=2c) align with chunks; odd blocks (m=2c+1) split into
  lower/upper triangle stationaries against chunk c (parts 64-127) and
  chunk c+1 (parts 0-63), PSUM-accumulated.  f-pairs pack 128 output
  partitions via tile_position.
Epilogue per f-pair: ACT abs-drain (PSUM->SBUF bf16), DVE mult by
  nrecipT (bf16 2x mode), DVE segmented reduce [128, 8b, 64m] -> [128, 8b].
Norms: block-ones stationary sums x^2 over c for a b-pair at once;
  sliding-window-64 via 6 doubling shift-adds in a [2b*64m, 127]-halo
  layout; 1/x on DVE, sqrt on ACT; PE-transpose into the [j, m] layout.
Finish: ones-contraction matmul over j, g-weighted fp sum, + bias sum.

All heavy matmuls run bf16 (1 col/cycle at any moving size; fp32r needs
>=256 cols).  x is cast to bf16 on the host (tolerance 2e-2; measured
error lands ~1e-3).
"""

import contextlib
import ctypes
import os
import sys
import types

import numpy as np

import concourse.bass as bass
import concourse.mybir as mybir
import concourse.tile as tile
from concourse.bass_utils import run_bass_kernel_spmd
from concourse.vector_clock import ScopedClock

F32 = mybir.dt.float32
F32R = mybir.dt.float32r
BF16 = mybir.dt.bfloat16

B, CIN, T = 64, 64, 4096
F, K = 128, 64
TOUT = T - K + 1          # 4033
NCORES = 8
BLOC = B // NCORES        # 8 batches per core
NCHUNK = T // 128         # 32 chunks of 128 time steps
SCALE = 64.0              # sqrt(CIN*K)

AF = mybir.ActivationFunctionType
ALU = mybir.AluOpType

from ml_dtypes import bfloat16 as np_bf16

# engine-assignment knobs (tuned against the sim cost model)
KNOB_SQ_ACT = 2      # of 8 sq-chunk drains per pair, every Nth on ACT
KNOB_Y_ACT = 2       # of 8 y-drain groups per b, every Nth on ACT
KNOB_MULT_GPS = 2    # of 64 f-pair mults, every Nth on GPS (rest DVE)
KNOB_SHIFT_GPS = True
KNOB_XSQ_GPS = True


# ---------------------------------------------------------------------------
# Container fixups: walrus here rejects >1 sem-wait on a Drain; TileContext's
# tail drain carries one wait per logical processor.  Chunk into single-wait
# drains.  Also recreate the (absent) antenv.axon_hooks NTFF profile hook so
# trace=True works when a test harness wants timings.
# ---------------------------------------------------------------------------

def _patched_drain_and_barrier(self, tick_clock, wait_clock):
    nc = self.nc
    drain_inst = nc.sync.drain()
    wait_clock.add_sem_waits(
        drain_inst.ins, ScopedClock({None: tick_clock.global_clock})
    )
    si = drain_inst.ins.sync_info
    waits = list(si.on_wait or []) if si else []
    if len(waits) > 1:
        si.on_wait = waits[:1]
        for w in waits[1:]:
            d2 = nc.sync.drain()
            si2 = d2.ins.sync_info
            if si2 is None:
                d2.ins.sync_info = mybir.SyncInfo(on_wait=[w], on_update=[])
            else:
                si2.on_wait = [w]
    nc.all_engine_barrier()
    assert self.sems is not None
    popped = nc._tile_sem_poison_stack.pop()
    assert popped is self._sem_poison
    nc.clear_and_free_semaphores(list(self.sems.allocated().values()))
    nc.all_engine_barrier()


def _install_ntff_hook():
    if "antenv.axon_hooks" in sys.modules:
        return
    try:
        lib = ctypes.CDLL("/opt/axon/libaxon_pjrt.so")
    except OSError:
        return
    if not hasattr(lib, "axon_start_nrt_profile"):
        return
    lib.axon_start_nrt_profile.argtypes = [
        ctypes.POINTER(ctypes.c_int64),
        ctypes.c_size_t,
    ]
    lib.axon_start_nrt_profile.restype = ctypes.c_int64
    lib.axon_stop_nrt_profile.argtypes = [ctypes.c_char_p]
    lib.axon_stop_nrt_profile.restype = ctypes.c_int64

    @contextlib.contextmanager
    def _hook(output_dir, device_ids):
        import jax

        jax.devices()
        if device_ids:
            ids = (ctypes.c_int64 * len(device_ids))(*device_ids)
            rc = lib.axon_start_nrt_profile(ids, len(device_ids))
        else:
            rc = lib.axon_start_nrt_profile(None, 0)
        if rc != 0:
            raise RuntimeError(f"axon_start_nrt_profile rc={rc}")
        try:
            yield
        finally:
            n = lib.axon_stop_nrt_profile(str(output_dir).encode())
            print(f"profile: {n} ntff file(s) in {output_dir}", file=sys.stderr)

    mod = types.ModuleType("antenv.axon_hooks")
    mod.get_axon_ntff_profile_hook = lambda: _hook
    mod.set_axon_ntff_profile_hook = lambda h: None
    import antenv

    antenv.axon_hooks = mod
    sys.modules["antenv.axon_hooks"] = mod


_ORIG_COMMIT = tile.TileContext._commit_instruction


def _commit_split_waits(self, inst, lazy_reg_writes=True):
    """walrus here allows only one sem-wait per instruction; move extras
    onto same-engine NOPs committed immediately before the instruction."""
    si = getattr(inst, "sync_info", None)
    if (
        si is not None
        and si.on_wait
        and len(si.on_wait) > 1
        and inst.engine != mybir.EngineType.Unassigned
    ):
        waits = list(si.on_wait)
        si.on_wait = waits[:1]
        for i, w in enumerate(waits[1:]):
            nop = mybir.InstNoOp(
                name=f"{inst.name}-wsplit{i}", ins=[], outs=[]
            )
            nop.engine = inst.engine
            nop.sync_info = mybir.SyncInfo(on_wait=[w], on_update=[])
            _ORIG_COMMIT(self, nop, lazy_reg_writes=False)
    return _ORIG_COMMIT(self, inst, lazy_reg_writes)


def install_fixups():
    tile.TileContext._drain_and_barrier = _patched_drain_and_barrier
    tile.TileContext._commit_instruction = _commit_split_waits
    _install_ntff_hook()


# ---------------------------------------------------------------------------
# Device program (identical on all 8 cores; inputs differ per core)
# ---------------------------------------------------------------------------

def build_program() -> bass.Bass:
    install_fixups()
    nc = bass.Bass()

    xs_in = nc.dram_tensor("xs", [BLOC, CIN, T], BF16, kind="ExternalInput")
    spat2_in = nc.dram_tensor("spat2", [128, F], BF16, kind="ExternalInput")
    a1_in = nc.dram_tensor("a1", [128, F, 128], BF16, kind="ExternalInput")
    a2_in = nc.dram_tensor("a2", [128, F, 128], BF16, kind="ExternalInput")
    e2_in = nc.dram_tensor("e2", [128, 32], BF16, kind="ExternalInput")
    id2_in = nc.dram_tensor("id2", [128, 64], BF16, kind="ExternalInput")
    ones1c_in = nc.dram_tensor("ones1c", [128, 1], F32, kind="ExternalInput")
    g1rep_in = nc.dram_tensor("g1rep", [1, 2, BLOC, 64], F32, kind="ExternalInput")
    bsum_in = nc.dram_tensor("bsum", [1, 1], F32, kind="ExternalInput")
    msk_in = nc.dram_tensor("msk", [128, 64], F32, kind="ExternalInput")
    out_d = nc.dram_tensor("out", [1, BLOC], F32, kind="ExternalOutput")

    PS = bass.MemorySpace.PSUM

    with tile.TileContext(nc) as tc:
        with (
            tc.tile_pool(name="const", bufs=1) as constp,
            tc.tile_pool(name="xp", bufs=2) as xp,
            tc.tile_pool(name="xsqp", bufs=2) as xsqp,
            tc.tile_pool(name="sqsbp", bufs=1) as sqsbp,
            tc.tile_pool(name="sq4p", bufs=4) as sq4p,
            tc.tile_pool(name="up", bufs=3) as up,
            tc.tile_pool(name="prodp", bufs=3) as prodp,
            tc.tile_pool(name="yps", bufs=2, space=PS) as yps,
            tc.tile_pool(name="wps", bufs=2, space=PS) as wps,
        ):
            # ---- constants -------------------------------------------------
            spat2 = constp.tile([128, F], BF16)
            nc.sync.dma_start(spat2[:], spat2_in[:])
            a1 = constp.tile([128, F, 128], BF16)
            a2 = constp.tile([128, F, 128], BF16)
            e2 = constp.tile([128, 32], BF16)
            nc.sync.dma_start(e2[:], e2_in[:])
            id2 = constp.tile([128, 64], BF16)
            nc.sync.dma_start(id2[:], id2_in[:])
            ones1c = constp.tile([128, 1], F32R)
            g1rep = constp.tile([1, 2, BLOC, 64], F32)
            bsumsb = constp.tile([1, 1], F32)
            msk = constp.tile([128, 64], F32)
            nc.sync.dma_start(msk[:], msk_in[:])

            # y in chunk-transposed layout: [i, b, f, c], bf16
            Y2 = constp.tile([128, BLOC, F, NCHUNK], BF16)
            # nrecip in [(eo, j), (fpar, b, c)] layout, fpar-duplicated
            nrecipT = constp.tile([128, 2, BLOC, 32], BF16)
            # per-(f,b) |cos| sums: [p=(eo,j), fpar, b, fp]
            acc = constp.tile([128, 2, BLOC, 64], F32R)

            # ---- phase A+B: norms + stage 1, per b-pair -------------------
            _PHASE = os.environ.get("V2_PHASE", "full")
            for bp in range(0 if _PHASE == "none" else BLOC // 2):
                b0, b1 = 2 * bp, 2 * bp + 1
                x2 = xp.tile([128, T], BF16, tag="x2")
                nc.sync.dma_start(x2[:], xs_in[b0:b0 + 2])

                # x^2 (gpsimd; SBUF-only engine, otherwise idle)
                xsq2 = xsqp.tile([128, T], BF16, tag="xsq")
                if KNOB_XSQ_GPS:
                    nc.gpsimd.tensor_tensor(xsq2[:], x2[:], x2[:], op=ALU.mult)
                else:
                    nc.scalar.square(xsq2[:], x2[:])

                # sq[b, t] = sum_c x^2: block-ones stationary, both b at
                # once; per-chunk PSUM banks drained straight into the
                # contiguous sq row (no DMA; psum is not DMA-readable here).
                sq_sb = sqsbp.tile([2, T + 128], F32, tag="sqsb")
                nc.vector.memset(sq_sb[:, T:], 0.0)
                for k in range(T // 512):
                    pk = wps.tile([32, 512], F32, tag="w")
                    nc.tensor.matmul(pk[:], e2[:], xsq2[:, 512 * k:512 * (k + 1)])
                    dst = sq_sb[:, 512 * k:512 * (k + 1)]
                    if k % KNOB_SQ_ACT == 0:
                        nc.scalar.copy(dst, pk[0:2, :])
                    else:
                        nc.vector.tensor_copy(dst, pk[0:2, :])

                # halo layout [2b x 64m, 127]: row (64h+m) = sq[b, 64m:64m+127]
                sq4h = sq4p.tile([128, 127], F32, tag="s4")
                nc.sync.dma_start(
                    sq4h[:, 0:64],
                    sq_sb[:, 0:T].rearrange("p (m j) -> p m j", m=64, j=64),
                )
                nc.sync.dma_start(
                    sq4h[:, 64:127],
                    sq_sb[:, 64:64 + T].rearrange(
                        "p (m j) -> p m j", m=64, j=64
                    )[:, :, 0:63],
                )
                # sliding-window-64 sum via doubling shifts
                cur = sq4h
                width = 127
                for sh in (1, 2, 4, 8, 16, 32):
                    width -= sh
                    nxt = sq4p.tile([128, 127], F32, tag="s4")
                    seng = nc.gpsimd if KNOB_SHIFT_GPS else nc.vector
                    seng.tensor_tensor(
                        nxt[:, 0:width], cur[:, 0:width], cur[:, sh:sh + width],
                        op=ALU.add,
                    )
                    cur = nxt
                assert width == 64
                rec4 = sq4p.tile([128, 127], F32, tag="s4")
                nc.vector.reciprocal(rec4[:, 0:64], cur[:, 0:64])
                # mask invalid tail (t > 4032): block m=63, j >= 1 -> 0
                recm = sq4p.tile([128, 127], F32, tag="s4")
                nc.vector.tensor_tensor(
                    recm[:, 0:64], rec4[:, 0:64], msk[:], op=ALU.mult
                )
                nrec4 = sq4p.tile([128, 64], BF16, tag="n4")
                nc.scalar.activation(nrec4[:], recm[:, 0:64], AF.Sqrt)

                # transpose [m, j] -> [j, m] per b on both halves, then
                # split even/odd blocks onto the (eo) partition halves
                for h in (0, 1):
                    ntp = wps.tile([128, 1024], BF16, tag="w")
                    for d in (0, 1):
                        nc.tensor.transpose(
                            ntp[64 * d:64 * (d + 1), 0:64],
                            nrec4[64 * h:64 * (h + 1), :],
                            id2[64 * h:64 * (h + 1), :],
                            tile_position=(64 * h, 64 * d),
                        )
                    b = 2 * bp + h
                    for fpar in (0, 1):
                        nc.scalar.copy(
                            nrecipT[0:64, fpar, b, :], ntp[0:64, 0:64:2]
                        )
                        nc.scalar.copy(
                            nrecipT[64:128, fpar, b, :], ntp[64:128, 1:64:2]
                        )

                # ---- stage 1: y chunks, time on partitions ----------------
                # stage-1 chunks for both halves interleaved: consecutive
                # matmuls alternate PE row-groups (64h), so each LDWEIGHTS
                # overlaps the other group's matmul.
                for grp in range(0 if _PHASE == "A" else NCHUNK // 4):
                    yp0 = yps.tile([128, 512], F32, tag="y")
                    yp1 = yps.tile([128, 512], F32, tag="y")
                    yph = [yp0, yp1]
                    for cc in range(4):
                        c = 4 * grp + cc
                        for h in (0, 1):
                            nc.tensor.matmul(
                                yph[h][:, 128 * cc:128 * (cc + 1)],
                                x2[64 * h:64 * (h + 1), 128 * c:128 * (c + 1)],
                                spat2[64 * h:64 * (h + 1), :],
                                tile_position=(64 * h, 0),
                                start=(cc == 0), stop=(cc == 3),
                                skip_group_check=True,
                            )
                    for h in (0, 1):
                        b = 2 * bp + h
                        ysrc = yph[h][:].rearrange("p (c f) -> p f c", c=4, f=F)
                        ydst = Y2[:, b, :, 4 * grp:4 * (grp + 1)]
                        if h == 0:
                            nc.scalar.copy(ydst, ysrc)
                        else:
                            nc.vector.tensor_copy(ydst, ysrc)

            _FULL = _PHASE == "full"
            # big phase-C constants: loaded here so their transfers overlap
            # phase A/B compute instead of serializing the prologue
            nc.sync.dma_start(a1[:], a1_in[:])
            nc.sync.dma_start(a2[:], a2_in[:])
            nc.sync.dma_start(ones1c[:], ones1c_in[:].bitcast(F32R))
            nc.sync.dma_start(g1rep[:], g1rep_in[:])
            nc.sync.dma_start(bsumsb[:], bsum_in[:])

            # ---- phase C: temporal conv + epilogue per f-pair -------------
            _SKIP_C = _PHASE in ("A", "AB")
            _SKIP_EPI = _PHASE == "ABnoepi"
            _SKIP_RED = _PHASE == "ABnored"
            for fp in range(0 if _SKIP_C else F // 2):
                pcv = wps.tile([128, 1024], F32, tag="w")
                pfv = [
                    pcv[:, 0:256].rearrange("p (b c) -> p b c", b=BLOC, c=32),
                    pcv[:, 512:768].rearrange("p (b c) -> p b c", b=BLOC, c=32),
                ]
                for s in (0, 1):
                    f = 2 * fp + s
                    # fused stationary [aev | aodl]: even blocks land on
                    # partitions 0-63 (j even-phase) and odd-lo on 64-127,
                    # in one 256-col matmul (shared moving operand)
                    nc.tensor.matmul(
                        pfv[s][:, :, :],
                        a1[:, f, :],
                        Y2[:, :, f, :],
                        start=True, stop=False,
                        skip_group_check=True,
                    )
                    # upper triangle vs chunk c+1 accumulates into the odd
                    # half; cols 0-63 of a2 are zero (adds 0 to even rows).
                    # Per-b: the sim requires collapsible 2D psum out APs.
                    for b in range(BLOC):
                        nc.tensor.matmul(
                            pfv[s][:, b, 0:31],
                            a2[:, f, :],
                            Y2[:, b, f, 1:32],
                            start=False, stop=(b == BLOC - 1),
                            skip_group_check=True,
                        )
                if _SKIP_EPI:
                    continue
                u = up.tile([128, 2, BLOC, 32], BF16, tag="u")
                psrc = (
                    pcv[:].rearrange("p (eo w) -> p eo w", eo=2, w=512)
                    [:, :, 0:256]
                    .rearrange("p eo (b c) -> p eo b c", b=BLOC, c=32)
                )
                nc.scalar.activation(u[:], psrc, AF.Abs)
                prod = prodp.tile([128, 2, BLOC, 32], BF16, tag="pr")
                meng = nc.gpsimd if fp % KNOB_MULT_GPS == 1 else nc.vector
                meng.tensor_tensor(
                    prod[:], u[:], nrecipT[:], op=ALU.mult
                )
                if _SKIP_RED:
                    continue
                with nc.allow_low_precision("f32r accumulate is full fp32"):
                    nc.vector.reduce_sum(
                        acc[:, :, :, fp], prod[:],
                        axis=mybir.AxisListType.X,
                    )

            # ---- finish ---------------------------------------------------
            if not _FULL:
                out_sb0 = constp.tile([1, BLOC], F32)
                nc.scalar.add(
                    out_sb0[:],
                    bsumsb[0:1, 0:1].to_broadcast([1, BLOC]),
                    0.0,
                )
                nc.sync.dma_start(out_d[:], out_sb0[:])
            t1s = constp.tile([1, 2 * BLOC * 64], F32)
            for half in ((0, 1) if _FULL else ()):
                t1 = wps.tile([1, 512], F32, tag="w")
                nc.tensor.matmul(
                    t1[:], ones1c[:],
                    acc[:].rearrange("p q b m -> p (q b m)")
                    [:, 512 * half:512 * (half + 1)],
                )
                nc.scalar.copy(t1s[:, 512 * half:512 * (half + 1)], t1[:])
            if _FULL:
                t2 = constp.tile([1, 2, BLOC, 64], F32)
                nc.vector.tensor_tensor(
                    t2[:],
                    t1s[:].rearrange("p (q b m) -> p q b m", q=2, b=BLOC, m=64),
                    g1rep[:],
                    op=ALU.mult,
                )
                t3 = constp.tile([1, 2, BLOC], F32)
                nc.vector.reduce_sum(t3[:], t2[:], axis=mybir.AxisListType.X)
                t4 = constp.tile([1, BLOC], F32)
                nc.vector.reduce_sum(
                    t4[:], t3[:].transpose([0, 2, 1]),
                    axis=mybir.AxisListType.X,
                )
                out_sb = constp.tile([1, BLOC], F32)
                nc.scalar.add(out_sb[:], t4[:], bsumsb[0:1, 0:1])
                nc.sync.dma_start(out_d[:], out_sb[:])

    return nc


_PROGRAM: bass.Bass | None = None


def _get_program() -> bass.Bass:
    global _PROGRAM
    if _PROGRAM is None:
        _PROGRAM = build_program()
    return _PROGRAM


# ---------------------------------------------------------------------------
# Host entry point
# ---------------------------------------------------------------------------

def host_params(conv_weights, spat_weights, weight, bias):
    """Tiny host-side precomputation of stationaries and scalars."""
    conv = np.asarray(conv_weights, dtype=np.float64)
    spat = np.asarray(spat_weights, dtype=np.float64)
    w = np.asarray(weight, dtype=np.float64)
    bb = np.asarray(bias, dtype=np.float64)

    wf = conv.astype(np.float32)            # [F, K]
    # spat^T duplicated on both partition halves
    spat2 = np.zeros((128, F), np.float32)
    spat2[0:64] = spat.T
    spat2[64:128] = spat.T

    # Fused Toeplitz stationaries, out partition p' = 64*eo + j:
    #   a1 cols 0:64  (eo=0): aev[i, f, j] = w[f, i-j] for 0 <= i-j < 64
    #   a1 cols 64:128 (eo=1): lo triangle, rows 64+v: w[f, v-j] for v >= j
    #   a2 cols 64:128 (eo=1): hi triangle, rows p:    w[f, p+64-j] for p < j
    # (zero-padded so both matmuls share one tile position / group)
    a1 = np.zeros((128, F, 128), np.float32)
    a2 = np.zeros((128, F, 128), np.float32)
    for j in range(64):
        a1[j:j + 64, :, j] = wf.T
        for p in range(j):
            a2[p, :, 64 + j] = wf[:, p + 64 - j]
        for v in range(j, 64):
            a1[64 + v, :, 64 + j] = wf[:, v - j]

    e2 = np.zeros((128, 32), np.float32)
    e2[0:64, 0] = 1.0
    e2[64:128, 1] = 1.0
    id2 = np.zeros((128, 64), np.float32)
    id2[0:64] = np.eye(64)
    id2[64:128] = np.eye(64)
    ones1c = np.ones((128, 1), np.float32)

    norm_w = np.sqrt((spat * spat).sum(1) * (conv * conv).sum(1))  # [F]
    g = (SCALE / (TOUT * norm_w) * w).astype(np.float32)           # [F]
    # g1rep[0, fpar, b, fp] = g[2*fp + fpar]
    g1rep = np.zeros((1, 2, BLOC, 64), np.float32)
    for s in (0, 1):
        g1rep[0, s, :, :] = g[2 * np.arange(64) + s][None, :]
    bsum = np.array([[bb.sum()]], dtype=np.float32)
    # mask: zero at (m = 63, j >= 1) i.e. partitions 63 and 127, cols 1:
    mskt = np.ones((128, 64), np.float32)
    mskt[63, 1:] = 0.0
    mskt[127, 1:] = 0.0

    def cast(a):
        return np.ascontiguousarray(a.astype(np_bf16))

    return {
        "spat2": cast(spat2),
        "a1": cast(a1),
        "a2": cast(a2),
        "e2": cast(e2),
        "id2": cast(id2),
        "ones1c": np.ascontiguousarray(ones1c),
        "g1rep": np.ascontiguousarray(g1rep),
        "bsum": bsum,
        "msk": np.ascontiguousarray(mskt),
    }


def kernel(x, conv_weights, spat_weights, weight, bias):
    params = host_params(conv_weights, spat_weights, weight, bias)
    xb = np.asarray(x, dtype=np.float32).astype(np_bf16)

    nc = _get_program()
    in_maps = []
    for c in range(NCORES):
        m = {"xs": np.ascontiguousarray(xb[c * BLOC:(c + 1) * BLOC])}
        m.update(params)
        in_maps.append(m)
    res = run_bass_kernel_spmd(nc, in_maps, core_ids=list(range(NCORES)))
    out = np.concatenate(
        [res.results[c]["out"].reshape(BLOC) for c in range(NCORES)]
    )
    return out.astype(np.float32)
